# revision 3
# baseline (speedup 1.0000x reference)
"""MoE grouped linear (DMoELinear) on 8 Trainium2 NeuronCores.

Expert-parallel sharding: tokens are sorted by expert id, so expert e's
tokens form one contiguous slice. Core e receives expert e's tokens
(padded to a uniform capacity C = max group size, so all cores run one
SPMD NEFF), expert e's weight and bias, and computes
    yT_e = (x_e @ W_e.T).T.bf16 + b_e.bf16
with the weight block as the stationary matmul operand and tokens as
the moving free dim. The weight is laid out k-major ([8 k-blocks] x
[16 dout blocks] x [128k x 128dout]) so one DMA per k-block delivers
the slices for all 16 output blocks in PE consumption order. The
matmul schedule runs a 4-wide output-block prologue over the first two
free-dim chunks, paced to DMA arrival, then drains output blocks
sequentially; the 58-wide tail chunks accumulate into shared PSUM
banks (8 blocks per bank). Bias adds fuse into DVE PSUM-evacuation
ops. The host does all routing/gather in numpy.
"""

import numpy as np
import ml_dtypes

N_TOK, D_IN, D_OUT, N_EXP = 8192, 1024, 2048, 8
N_CORES = 8
P = 128
NFREE = 512  # max matmul moving free dim (one PSUM bank of f32)

BF16 = ml_dtypes.bfloat16

_nc_cache: dict[int, object] = {}


def _build_bass(C: int):
    """Emit the per-core Bass/Tile kernel for token capacity C."""
    import concourse.bass as bass  # noqa: F401  (registers engines)
    import concourse.mybir as mybir
    import concourse.tile as tile
    from concourse import bacc

    dt = mybir.dt
    KT = D_IN // P      # 8 contraction tiles
    DB = D_OUT // P     # 16 output-row blocks
    c2w = C - 2 * NFREE  # tail chunk width (58 for C=1082)
    assert 0 < c2w <= NFREE
    PRO = 4             # prologue interleave width (db blocks)

    nc = bacc.Bacc("TRN2", target_bir_lowering=False)

    xT_d = nc.dram_tensor("xT", [D_IN, C], dt.bfloat16, kind="ExternalInput")
    # k-major weight: row (ki*128+p) holds cols (db*128+d) = W[db*128+d,
    # ki*128+p]; one 128-row block per ki covers every db.
    w_d = nc.dram_tensor("wkm", [D_IN, D_OUT], dt.bfloat16, kind="ExternalInput")
    bias_d = nc.dram_tensor("biasp", [P, DB], dt.float32, kind="ExternalInput")
    y_d = nc.dram_tensor("yT", [D_OUT, C], dt.bfloat16, kind="ExternalOutput")

    HDB = DB // 2  # dbs per w half-block

    with tile.TileContext(nc) as tc:
        with (
            tc.tile_pool(name="persist", bufs=1) as ppool,
            tc.tile_pool(name="yout", bufs=5) as ypool,
            tc.tile_pool(name="psum", bufs=8, space="PSUM") as pspool,
        ):
            # SBUF operand tiles. wa holds dbs 0-7, wb dbs 8-15; within
            # each, cols (ki*1024 + (db%8)*128 + d).
            wa = ppool.tile([P, KT * HDB * P], dt.bfloat16, name="wa", tag="wa")
            wb = ppool.tile([P, KT * HDB * P], dt.bfloat16, name="wb", tag="wb")
            x_tiles = [
                ppool.tile([P, C], dt.bfloat16, name=f"x{ki}", tag=f"x{ki}")
                for ki in range(KT)
            ]
            bt = ppool.tile([P, DB], dt.float32, name="bias", tag="bias")

            def lhsT(db, ki):
                t, d = (wa, db) if db < HDB else (wb, db - HDB)
                off = (ki * HDB + d) * P
                return t[:, off:off + P]

            # DMA emission order = arrival order. sync: w k-blocks (half
            # A then half B) then bias then y-outs; scalar: x chunk DMAs
            # in (ki, chunk) consumption order then y-outs.
            # First k-block leads both rings so the first matmul's
            # operands land ASAP.
            nc.sync.dma_start(
                wa[:, :HDB * P], w_d[:P, :HDB * P]
            )
            nc.scalar.dma_start(x_tiles[0][:, :NFREE], xT_d[:P, :NFREE])
            for ki in range(1, KT):
                nc.sync.dma_start(
                    wa[:, ki * HDB * P:(ki + 1) * HDB * P],
                    w_d[ki * P:(ki + 1) * P, :HDB * P],
                )
            nc.scalar.dma_start(x_tiles[0][:, NFREE:], xT_d[:P, NFREE:])
            for ki in range(1, KT):
                nc.scalar.dma_start(
                    x_tiles[ki][:, :NFREE], xT_d[ki * P:(ki + 1) * P, :NFREE]
                )
                nc.scalar.dma_start(
                    x_tiles[ki][:, NFREE:], xT_d[ki * P:(ki + 1) * P, NFREE:]
                )
            for ki in range(KT):
                nc.sync.dma_start(
                    wb[:, ki * HDB * P:(ki + 1) * HDB * P],
                    w_d[ki * P:(ki + 1) * P, HDB * P:],
                )
            nc.sync.dma_start(bt[:], bias_d[:])

            # Warm the PE's HAM clock gate with dummy matmuls on a zeroed
            # scratch tile while the first operand DMAs land (~3.4us of
            # PE activity flips the clock gate from 1.2GHz to 2.4GHz).
            warm = ppool.tile([P, P], dt.bfloat16, name="warm", tag="warm")
            nc.vector.memset(warm[:], 0.0)
            wps = pspool.tile([P, P], dt.float32, name="wps", tag="ps")
            for _ in range(16):
                nc.tensor.matmul(wps[:], warm[:], warm[:], start=True, stop=True)

            ps01 = {}

            def alloc01(db):
                ps01[db] = [
                    pspool.tile([P, NFREE], dt.float32, name=f"ps{db}_{j}", tag="ps")
                    for j in range(2)
                ]

            def mm01(db, ki):
                l = lhsT(db, ki)
                st, sp = ki == 0, ki == KT - 1
                nc.tensor.matmul(
                    ps01[db][0][:], l, x_tiles[ki][:, :NFREE], start=st, stop=sp
                )
                nc.tensor.matmul(
                    ps01[db][1][:], l, x_tiles[ki][:, NFREE:2 * NFREE],
                    start=st, stop=sp,
                )

            def mm2(db, ki, shared):
                off = (db % HDB) * c2w
                nc.tensor.matmul(
                    shared[:, off:off + c2w],
                    lhsT(db, ki),
                    x_tiles[ki][:, 2 * NFREE:],
                    start=(ki == 0),
                    stop=(ki == KT - 1),
                )

            ysbs = {}

            def evac01(db):
                ysb = ypool.tile([P, C], dt.bfloat16, name="ysb", tag="ysb")
                ysbs[db] = ysb
                bc = bt[:, db:db + 1]
                nc.vector.tensor_scalar_add(ysb[:, :NFREE], ps01[db][0][:], bc)
                nc.vector.tensor_scalar_add(
                    ysb[:, NFREE:2 * NFREE], ps01[db][1][:], bc
                )
                del ps01[db]

            def evac2_and_out(db, shared, last=False):
                ysb = ysbs.pop(db)
                off = (db % HDB) * c2w
                nc.vector.tensor_scalar_add(
                    ysb[:, 2 * NFREE:], shared[:, off:off + c2w], bt[:, db:db + 1]
                )
                if last:
                    half = C // 2
                    nc.sync.dma_start(y_d[db * P:(db + 1) * P, :half], ysb[:, :half])
                    nc.scalar.dma_start(
                        y_d[db * P:(db + 1) * P, half:], ysb[:, half:]
                    )
                else:
                    eng = nc.sync if db % 2 == 0 else nc.scalar
                    eng.dma_start(y_d[db * P:(db + 1) * P, :], ysb[:])

            # P1: 4-wide db interleave over chunks c0/c1, ki-major, so
            # each x tile and w k-block is consumed at DMA-arrival pace.
            for db in range(PRO):
                alloc01(db)
            for ki in range(KT):
                for db in range(PRO):
                    mm01(db, ki)
            for db in range(PRO):
                evac01(db)

            # P1.5: tail chunks of the prologue dbs into a shared bank.
            shared_a = pspool.tile([P, HDB * c2w], dt.float32, name="c2a", tag="ps")
            for db in range(PRO):
                for ki in range(KT):
                    mm2(db, ki, shared_a)
            for db in range(PRO):
                evac2_and_out(db, shared_a)

            # P2: remaining dbs sequentially; chunk c2 rides on the same
            # LDWEIGHTS as c0/c1 per (db, ki).
            shared_b = None
            for db in range(PRO, DB):
                shared = shared_a if db < HDB else shared_b
                if db >= HDB and shared_b is None:
                    shared_b = shared = pspool.tile(
                        [P, HDB * c2w], dt.float32, name="c2b", tag="ps"
                    )
                alloc01(db)
                for ki in range(KT):
                    l = lhsT(db, ki)
                    st, sp = ki == 0, ki == KT - 1
                    nc.tensor.matmul(
                        ps01[db][0][:], l, x_tiles[ki][:, :NFREE], start=st, stop=sp
                    )
                    nc.tensor.matmul(
                        ps01[db][1][:], l, x_tiles[ki][:, NFREE:2 * NFREE],
                        start=st, stop=sp,
                    )
                    mm2(db, ki, shared)
                evac01(db)
                evac2_and_out(db, shared, last=(db == DB - 1))

    nc.compile()
    return nc


def _run_spmd(in_maps, C, trace=False, trace_cores=None):
    from concourse.bass_utils import run_bass_kernel_spmd

    nc = _nc_cache.get(C)
    if nc is None:
        nc = _build_bass(C)
        _nc_cache[C] = nc
    return run_bass_kernel_spmd(
        nc,
        in_maps,
        core_ids=list(range(N_CORES)),
        trace=trace,
        trace_cores=trace_cores,
    )


def _prepare(x, weight, bias, ids_sorted):
    """Host-side routing: returns (in_maps, C, counts, starts)."""
    x = np.asarray(x)
    weight = np.asarray(weight)
    bias = np.asarray(bias)
    ids = np.asarray(ids_sorted)

    counts = np.bincount(ids, minlength=N_EXP).astype(np.int64)
    starts = np.zeros(N_EXP, dtype=np.int64)
    starts[1:] = np.cumsum(counts)[:-1]
    C = max(int(counts.max()), 2 * NFREE + 2)
    C += C % 2  # keep the half-split even

    xb = x.astype(BF16)
    in_maps = []
    for e in range(N_EXP):
        n_e = int(counts[e])
        xeT = np.zeros((D_IN, C), dtype=BF16)
        if n_e:
            xeT[:, :n_e] = xb[starts[e]:starts[e] + n_e].T
        # k-major weight: row (ki*128+p), col (db*128+d) = W[db*128+d, ki*128+p]
        weT = weight[e].T.astype(BF16)  # [d_in, d_out]
        wkm = np.ascontiguousarray(weT)
        bp = np.ascontiguousarray(
            bias[e].astype(BF16).astype(np.float32).reshape(D_OUT // P, P).T
        )
        in_maps.append({"xT": xeT, "wkm": wkm, "biasp": bp})
    return in_maps, C, counts, starts


def _assemble(results, counts, starts):
    out = np.empty((N_TOK, D_OUT), dtype=BF16)
    for e in range(N_EXP):
        n_e = int(counts[e])
        if n_e:
            out[starts[e]:starts[e] + n_e] = results[e]["yT"][:, :n_e].T
    return out


def kernel(x, weight, bias, ids_sorted):
    in_maps, C, counts, starts = _prepare(x, weight, bias, ids_sorted)
    res = _run_spmd(in_maps, C)
    return _assemble(res.results, counts, starts)


# revision 7
# speedup vs baseline: 1.0157x; 1.0157x over previous
"""MoE grouped linear (DMoELinear) on 8 Trainium2 NeuronCores.

Expert-parallel sharding: tokens are sorted by expert id, so expert e's
tokens form one contiguous slice. Core e receives expert e's tokens
(padded to a uniform capacity C = max group size, so all cores run one
SPMD NEFF), expert e's weight and bias, and computes
    yT_e = (x_e @ W_e.T).T.bf16 + b_e.bf16
with the weight block as the stationary matmul operand and tokens as
the moving free dim. The weight is laid out k-major ([8 k-blocks] x
[16 dout blocks] x [128k x 128dout]) so one DMA per k-block delivers
the slices for all 16 output blocks in PE consumption order. The
matmul schedule runs a 4-wide output-block prologue over the first two
free-dim chunks, paced to DMA arrival, then drains output blocks
sequentially; the 58-wide tail chunks accumulate into shared PSUM
banks (8 blocks per bank). Bias adds fuse into DVE PSUM-evacuation
ops. The host does all routing/gather in numpy.
"""

import numpy as np
import ml_dtypes

N_TOK, D_IN, D_OUT, N_EXP = 8192, 1024, 2048, 8
N_CORES = 8
P = 128
NFREE = 512  # max matmul moving free dim (one PSUM bank of f32)

BF16 = ml_dtypes.bfloat16

_nc_cache: dict[int, object] = {}


def _build_bass(C: int):
    """Emit the per-core Bass/Tile kernel for token capacity C."""
    import concourse.bass as bass  # noqa: F401  (registers engines)
    import concourse.mybir as mybir
    import concourse.tile as tile
    from concourse import bacc

    dt = mybir.dt
    KT = D_IN // P      # 8 contraction tiles
    DB = D_OUT // P     # 16 output-row blocks
    c2w = C - 2 * NFREE  # tail chunk width (58 for C=1082)
    assert 0 < c2w <= NFREE
    PRO = 3             # prologue interleave width (db blocks)

    nc = bacc.Bacc("TRN2", target_bir_lowering=False)

    xT_d = nc.dram_tensor("xT", [D_IN, C], dt.bfloat16, kind="ExternalInput")
    # k-major weight: row (ki*128+p) holds cols (db*128+d) = W[db*128+d,
    # ki*128+p]; one 128-row block per ki covers every db.
    w_d = nc.dram_tensor("wkm", [D_IN, D_OUT], dt.bfloat16, kind="ExternalInput")
    bias_d = nc.dram_tensor("biasp", [P, DB], dt.float32, kind="ExternalInput")
    y_d = nc.dram_tensor("yT", [D_OUT, C], dt.bfloat16, kind="ExternalOutput")

    HDB = DB // 2  # dbs per w half-block

    with tile.TileContext(nc) as tc:
        with (
            tc.tile_pool(name="persist", bufs=1) as ppool,
            tc.tile_pool(name="yout", bufs=5) as ypool,
            tc.tile_pool(name="psum", bufs=8, space="PSUM") as pspool,
        ):
            # SBUF operand tiles. wa holds dbs 0-7, wb dbs 8-15; within
            # each, cols (ki*1024 + (db%8)*128 + d).
            wa = ppool.tile([P, KT * HDB * P], dt.bfloat16, name="wa", tag="wa")
            wb = ppool.tile([P, KT * HDB * P], dt.bfloat16, name="wb", tag="wb")
            x_tiles = [
                ppool.tile([P, C], dt.bfloat16, name=f"x{ki}", tag=f"x{ki}")
                for ki in range(KT)
            ]
            bt = ppool.tile([P, DB], dt.float32, name="bias", tag="bias")

            def lhsT(db, ki):
                t, d = (wa, db) if db < HDB else (wb, db - HDB)
                off = (ki * HDB + d) * P
                return t[:, off:off + P]

            # DMA emission order = arrival order. sync: first k-block +
            # first x chunk (critical path for the first matmul, kept
            # away from scalar's hoisted ACT_TABLE_LOAD), remaining w
            # k-blocks, bias; scalar: x chunk DMAs in (ki, chunk)
            # consumption order. y-outs alternate later.
            nc.sync.dma_start(wa[:, :HDB * P], w_d[:P, :HDB * P])
            nc.sync.dma_start(x_tiles[0][:, :NFREE], xT_d[:P, :NFREE])
            nc.scalar.dma_start(x_tiles[0][:, NFREE:], xT_d[:P, NFREE:])
            for ki in range(1, KT):
                nc.sync.dma_start(
                    wa[:, ki * HDB * P:(ki + 1) * HDB * P],
                    w_d[ki * P:(ki + 1) * P, :HDB * P],
                )
                nc.scalar.dma_start(
                    x_tiles[ki][:, :NFREE], xT_d[ki * P:(ki + 1) * P, :NFREE]
                )
                nc.scalar.dma_start(
                    x_tiles[ki][:, NFREE:], xT_d[ki * P:(ki + 1) * P, NFREE:]
                )
            for ki in range(KT):
                nc.sync.dma_start(
                    wb[:, ki * HDB * P:(ki + 1) * HDB * P],
                    w_d[ki * P:(ki + 1) * P, HDB * P:],
                )
            nc.sync.dma_start(bt[:], bias_d[:])

            # Warm the PE's HAM clock gate with dummy matmuls on a zeroed
            # scratch tile while the first operand DMAs land (~3.4us of
            # PE activity flips the clock gate from 1.2GHz to 2.4GHz).
            warm = ppool.tile([P, P], dt.bfloat16, name="warm", tag="warm")
            nc.vector.memset(warm[:], 0.0)
            wps = pspool.tile([P, P], dt.float32, name="wps", tag="ps")
            for _ in range(30):
                nc.tensor.matmul(wps[:], warm[:], warm[:], start=True, stop=True)

            ps01 = {}

            def alloc01(db):
                ps01[db] = [
                    pspool.tile([P, NFREE], dt.float32, name=f"ps{db}_{j}", tag="ps")
                    for j in range(2)
                ]

            def mm01(db, ki):
                l = lhsT(db, ki)
                st, sp = ki == 0, ki == KT - 1
                nc.tensor.matmul(
                    ps01[db][0][:], l, x_tiles[ki][:, :NFREE], start=st, stop=sp
                )
                nc.tensor.matmul(
                    ps01[db][1][:], l, x_tiles[ki][:, NFREE:2 * NFREE],
                    start=st, stop=sp,
                )

            def mm2(db, ki, shared):
                off = (db % HDB) * c2w
                nc.tensor.matmul(
                    shared[:, off:off + c2w],
                    lhsT(db, ki),
                    x_tiles[ki][:, 2 * NFREE:],
                    start=(ki == 0),
                    stop=(ki == KT - 1),
                )

            ysbs = {}
            ep = [0]

            def ev(dst, src, bc):
                # alternate PSUM evacuation between ACT and DVE so
                # neither engine's serial throughput gates the pipeline
                if ep[0] % 2 == 0:
                    nc.scalar.add(dst, src, bc)
                else:
                    nc.vector.tensor_scalar_add(dst, src, bc)
                ep[0] += 1

            def evac01(db):
                ysb = ypool.tile([P, C], dt.bfloat16, name="ysb", tag="ysb")
                ysbs[db] = ysb
                bc = bt[:, db:db + 1]
                ev(ysb[:, :NFREE], ps01[db][0][:], bc)
                ev(ysb[:, NFREE:2 * NFREE], ps01[db][1][:], bc)
                del ps01[db]

            def evac2_and_out(db, shared, last=False):
                ysb = ysbs.pop(db)
                off = (db % HDB) * c2w
                ev(ysb[:, 2 * NFREE:], shared[:, off:off + c2w], bt[:, db:db + 1])
                if last:
                    # three pieces, each released as soon as its chunk's
                    # evacuation lands, so the final transfer is tiny
                    nc.sync.dma_start(
                        y_d[db * P:(db + 1) * P, :NFREE], ysb[:, :NFREE]
                    )
                    nc.scalar.dma_start(
                        y_d[db * P:(db + 1) * P, NFREE:2 * NFREE],
                        ysb[:, NFREE:2 * NFREE],
                    )
                    nc.sync.dma_start(
                        y_d[db * P:(db + 1) * P, 2 * NFREE:], ysb[:, 2 * NFREE:]
                    )
                else:
                    eng = nc.sync if db % 2 == 0 else nc.scalar
                    eng.dma_start(y_d[db * P:(db + 1) * P, :], ysb[:])

            # P1: 4-wide db interleave over chunks c0/c1, ki-major, so
            # each x tile and w k-block is consumed at DMA-arrival pace.
            for db in range(PRO):
                alloc01(db)
            for ki in range(KT):
                for db in range(PRO):
                    mm01(db, ki)
            for db in range(PRO):
                evac01(db)

            # P1.5: tail chunks of the prologue dbs into a shared bank.
            shared_a = pspool.tile([P, HDB * c2w], dt.float32, name="c2a", tag="ps")
            for db in range(PRO):
                for ki in range(KT):
                    mm2(db, ki, shared_a)
            for db in range(PRO):
                evac2_and_out(db, shared_a)

            # P2: remaining dbs sequentially; chunk c2 rides on the same
            # LDWEIGHTS as c0/c1 per (db, ki).
            shared_b = None
            for db in range(PRO, DB):
                shared = shared_a if db < HDB else shared_b
                if db >= HDB and shared_b is None:
                    shared_b = shared = pspool.tile(
                        [P, HDB * c2w], dt.float32, name="c2b", tag="ps"
                    )
                alloc01(db)
                for ki in range(KT):
                    l = lhsT(db, ki)
                    st, sp = ki == 0, ki == KT - 1
                    nc.tensor.matmul(
                        ps01[db][0][:], l, x_tiles[ki][:, :NFREE], start=st, stop=sp
                    )
                    nc.tensor.matmul(
                        ps01[db][1][:], l, x_tiles[ki][:, NFREE:2 * NFREE],
                        start=st, stop=sp,
                    )
                    mm2(db, ki, shared)
                evac01(db)
                evac2_and_out(db, shared, last=(db == DB - 1))

    nc.compile()
    return nc


def _run_spmd(in_maps, C, trace=False, trace_cores=None):
    from concourse.bass_utils import run_bass_kernel_spmd

    nc = _nc_cache.get(C)
    if nc is None:
        nc = _build_bass(C)
        _nc_cache[C] = nc
    return run_bass_kernel_spmd(
        nc,
        in_maps,
        core_ids=list(range(N_CORES)),
        trace=trace,
        trace_cores=trace_cores,
    )


def _prepare(x, weight, bias, ids_sorted):
    """Host-side routing: returns (in_maps, C, counts, starts)."""
    x = np.asarray(x)
    weight = np.asarray(weight)
    bias = np.asarray(bias)
    ids = np.asarray(ids_sorted)

    counts = np.bincount(ids, minlength=N_EXP).astype(np.int64)
    starts = np.zeros(N_EXP, dtype=np.int64)
    starts[1:] = np.cumsum(counts)[:-1]
    C = max(int(counts.max()), 2 * NFREE + 2)
    C += C % 2  # keep the half-split even

    xb = x.astype(BF16)
    in_maps = []
    for e in range(N_EXP):
        n_e = int(counts[e])
        xeT = np.zeros((D_IN, C), dtype=BF16)
        if n_e:
            xeT[:, :n_e] = xb[starts[e]:starts[e] + n_e].T
        # k-major weight: row (ki*128+p), col (db*128+d) = W[db*128+d, ki*128+p]
        weT = weight[e].T.astype(BF16)  # [d_in, d_out]
        wkm = np.ascontiguousarray(weT)
        bp = np.ascontiguousarray(
            bias[e].astype(BF16).astype(np.float32).reshape(D_OUT // P, P).T
        )
        in_maps.append({"xT": xeT, "wkm": wkm, "biasp": bp})
    return in_maps, C, counts, starts


def _assemble(results, counts, starts):
    out = np.empty((N_TOK, D_OUT), dtype=BF16)
    for e in range(N_EXP):
        n_e = int(counts[e])
        if n_e:
            out[starts[e]:starts[e] + n_e] = results[e]["yT"][:, :n_e].T
    return out


def kernel(x, weight, bias, ids_sorted):
    in_maps, C, counts, starts = _prepare(x, weight, bias, ids_sorted)
    res = _run_spmd(in_maps, C)
    return _assemble(res.results, counts, starts)
